# revision 1
# baseline (speedup 1.0000x reference)
"""Cached-attention kernel for Trainium2 (8 NeuronCores, Bass/Tile).

Problem: B=4, L=2048 new tokens, S=2048 cached tokens, D=2048.
  Q = x @ Wq.T ; K = x @ Wk.T ; V = x @ Wv.T
  K_cal = concat(K, cache_k) ; V_cal = concat(V, cache_v)
  out = softmax(Q @ K_cal.T / sqrt(D)) @ V_cal

Sharding: 8 cores = (batch b in 0..3) x (key-half h in 0..1). Each core
handles ALL queries of its batch against HALF the keys (1024 cached +
1024 new); per-core K/V projections cover only its half of the new
tokens.  Softmax is computed flash-style without max subtraction
(scores are O(6) here, exp is safe in fp32): each core returns the
un-normalized numerator sum_p(s) * V (transposed, [D, L]) and the
denominator sum_p(s) [L]; the host combines the two halves exactly.

All matmuls run in float32r (TF32-like: ~1.5e-4 rms rel error, 4x the
throughput of fp32 on the PE array).  PSUM accumulation is fp32.

Layouts are arranged so no on-device transposes are needed: the host
feeds x^T, W^T and cache_k^T; the kernel emits the numerator
transposed and the host transposes back (free on host).
"""

import numpy as np

import concourse.bass as bass
import concourse.tile as tile
from concourse import bacc, mybir
from concourse import bass2jax

F32 = mybir.dt.float32
F32R = mybir.dt.float32r

D = 2048          # model dim (= projection output dim)
L = 2048          # new tokens (queries)
HALF = 1024       # per-core share of new tokens / cached tokens
NT = D // 128     # 16 tiles of 128 along D/E/L
SCALE = 1.0 / float(np.sqrt(D))
N_CORES = 8

_NC_CACHE = {}


def build_program(reps=1):
    key = ("nc", reps)
    if key in _NC_CACHE:
        return _NC_CACHE[key]
    nc = bacc.Bacc(None, target_bir_lowering=False, debug=False)
    xT = nc.dram_tensor("xT", [D, L], F32R, kind="ExternalInput")
    xkvT = nc.dram_tensor("xkvT", [D, HALF], F32R, kind="ExternalInput")
    wqT = nc.dram_tensor("wqT", [D, D], F32R, kind="ExternalInput")
    wkT = nc.dram_tensor("wkT", [D, D], F32R, kind="ExternalInput")
    wvT = nc.dram_tensor("wvT", [D, D], F32R, kind="ExternalInput")
    kcT = nc.dram_tensor("kcT", [D, HALF], F32R, kind="ExternalInput")
    vc = nc.dram_tensor("vc", [HALF, D], F32R, kind="ExternalInput")
    outT = nc.dram_tensor("outT", [D, L], F32, kind="ExternalOutput")
    den = nc.dram_tensor("den", [1, L], F32, kind="ExternalOutput")

    from contextlib import ExitStack
    with tile.TileContext(nc) as tc:
        with ExitStack() as _rep_stack:
            if reps > 1:
                _rep_stack.enter_context(
                    tc.For_i(0, reps, 1, hint_engines=tuple(mybir.EngineType))
                )
            _emit_body(nc, tc, xT, xkvT, wqT, wkT, wvT, kcT, vc, outT, den)
    nc.compile()
    _NC_CACHE[key] = nc
    return nc


def _emit_body(nc, tc, xT, xkvT, wqT, wkT, wvT, kcT, vc, outT, den):

    xT_r = xT.rearrange("(t p) l -> p t l", p=128)
    xkvT_r = xkvT.rearrange("(t p) s -> p t s", p=128)
    wqT_r = wqT.rearrange("(t p) e -> p t e", p=128)
    wkT_r = wkT.rearrange("(t p) e -> p t e", p=128)
    wvT_r = wvT.rearrange("(t p) e -> p t e", p=128)
    kcT_r = kcT.rearrange("(t p) s -> p t s", p=128)
    vc_r = vc.rearrange("(t p) d -> p t d", p=128)

    if True:
        with tc.tile_pool(name="dram", bufs=1, space="DRAM") as dpool:
            qt_d = dpool.tile([D, L], F32R, tag="qt")
            kt_d = dpool.tile([D, HALF], F32R, tag="kt")
            v_d = dpool.tile([HALF, D], F32R, tag="vd")
            qt_dr = qt_d[:].rearrange("(t p) l -> p t l", p=128)
            kt_dr = kt_d[:].rearrange("(t p) s -> p t s", p=128)
            v_dr = v_d[:].rearrange("(t p) d -> p t d", p=128)

            # ---------- Phase Q: QT[e, l] = Wq @ x^T ----------
            with (
                tc.tile_pool(name="xt", bufs=1) as xpool,
                tc.tile_pool(name="wq", bufs=3) as wpool,
                tc.tile_pool(name="qo", bufs=4) as opool,
                tc.tile_pool(name="psQ", bufs=4, space="PSUM") as pspool,
            ):
                # load x^T in 4 column chunks so the first matmuls start early
                xt_c = []
                for lc in range(4):
                    t = xpool.tile([128, NT, 512], F32R, tag=f"xt{lc}")
                    nc.sync.dma_start(t[:], xT_r[:, :, lc * 512:(lc + 1) * 512])
                    xt_c.append(t)
                for et in range(NT):
                    w_sb = wpool.tile([128, NT, 128], F32R, tag="w")
                    nc.sync.dma_start(w_sb[:], wqT_r[:, :, et * 128:(et + 1) * 128])
                    for lc in range(4):
                        ps = pspool.tile([128, 512], F32, tag="ps")
                        for dt in range(NT):
                            nc.tensor.matmul(
                                ps[:],
                                w_sb[:, dt, :],
                                xt_c[lc][:, dt, :],
                                start=(dt == 0),
                                stop=(dt == NT - 1),
                            )
                        o_sb = opool.tile([128, 512], F32R, tag="o")
                        nc.vector.tensor_copy(o_sb[:], ps[:])
                        nc.sync.dma_start(
                            qt_d[et * 128:(et + 1) * 128, lc * 512:(lc + 1) * 512],
                            o_sb[:],
                        )

            # ---------- Phase K/V: KT[e, s_new], V[s_new, d] ----------
            with (
                tc.tile_pool(name="xkv", bufs=1) as xkpool,
                tc.tile_pool(name="wk2", bufs=3) as wkpool,
                tc.tile_pool(name="wv2", bufs=2) as wvpool,
                tc.tile_pool(name="kvo", bufs=4) as kvopool,
                tc.tile_pool(name="psKV", bufs=4, space="PSUM") as pskv,
            ):
                xkv_c = []
                for sc in range(2):
                    t = xkpool.tile([128, NT, 512], F32R, tag=f"xkv{sc}")
                    nc.sync.dma_start(t[:], xkvT_r[:, :, sc * 512:(sc + 1) * 512])
                    xkv_c.append(t)
                # KT[e, s] = Wk @ xkv^T
                for et in range(NT):
                    w_sb = wkpool.tile([128, NT, 128], F32R, tag="wk")
                    nc.sync.dma_start(w_sb[:], wkT_r[:, :, et * 128:(et + 1) * 128])
                    for sc in range(2):
                        ps = pskv.tile([128, 512], F32, tag="ps")
                        for dt in range(NT):
                            nc.tensor.matmul(
                                ps[:],
                                w_sb[:, dt, :],
                                xkv_c[sc][:, dt, :],
                                start=(dt == 0),
                                stop=(dt == NT - 1),
                            )
                        o_sb = kvopool.tile([128, 512], F32R, tag="o")
                        nc.vector.tensor_copy(o_sb[:], ps[:])
                        nc.sync.dma_start(
                            kt_d[et * 128:(et + 1) * 128, sc * 512:(sc + 1) * 512],
                            o_sb[:],
                        )
                # V[s, d] = x_kv @ Wv^T  (natural layout; lhsT = xkv^T tiles)
                for dc in range(4):
                    wv_sb = wvpool.tile([128, NT, 512], F32R, tag="wv")
                    nc.sync.dma_start(wv_sb[:], wvT_r[:, :, dc * 512:(dc + 1) * 512])
                    for st in range(8):
                        sc, so = divmod(st, 4)
                        ps = pskv.tile([128, 512], F32, tag="ps")
                        for dt in range(NT):
                            nc.tensor.matmul(
                                ps[:],
                                xkv_c[sc][:, dt, so * 128:(so + 1) * 128],
                                wv_sb[:, dt, :],
                                start=(dt == 0),
                                stop=(dt == NT - 1),
                            )
                        o_sb = kvopool.tile([128, 512], F32R, tag="o")
                        nc.vector.tensor_copy(o_sb[:], ps[:])
                        nc.sync.dma_start(
                            v_d[st * 128:(st + 1) * 128, dc * 512:(dc + 1) * 512],
                            o_sb[:],
                        )

            # ---------- Phase A: attention ----------
            # local key axis: s-tiles 0..7 = cached half, 8..15 = new half
            with (
                tc.tile_pool(name="qt2", bufs=1) as qpool,
                tc.tile_pool(name="pT", bufs=1) as ppool,
                tc.tile_pool(name="kt2", bufs=4) as kpool,
                tc.tile_pool(name="v2", bufs=3) as vpool,
                tc.tile_pool(name="oA", bufs=4) as oApool,
                tc.tile_pool(name="cst", bufs=1) as cpool,
                tc.tile_pool(name="psS", bufs=3, space="PSUM") as psS,
                tc.tile_pool(name="psO", bufs=4, space="PSUM") as psO,
                tc.tile_pool(name="psD", bufs=1, space="PSUM") as psD,
            ):
                ones_f = cpool.tile([128, 1], F32, tag="ones_f")
                nc.gpsimd.memset(ones_f[:], 1.0)
                ones = cpool.tile([128, 1], F32R, tag="ones")
                nc.vector.tensor_copy(ones[:], ones_f[:])

                for lc2 in range(2):
                    lo = lc2 * HALF
                    qt_sb = qpool.tile([128, NT, HALF], F32R, tag="qt")
                    nc.sync.dma_start(qt_sb[:], qt_dr[:, :, lo:lo + HALF])
                    pT = ppool.tile([128, NT, HALF], F32R, tag="pT")

                    # scores^T [s, l] and p = exp(scale * s)
                    for st in range(NT):
                        kt_sb = kpool.tile([128, NT, 128], F32R, tag="kt")
                        if st < 8:
                            src = kcT_r[:, :, st * 128:(st + 1) * 128]
                        else:
                            src = kt_dr[:, :, (st - 8) * 128:(st - 7) * 128]
                        nc.sync.dma_start(kt_sb[:], src)
                        for ls in range(2):
                            ps = psS.tile([128, 512], F32, tag="psS")
                            for et in range(NT):
                                nc.tensor.matmul(
                                    ps[:],
                                    kt_sb[:, et, :],
                                    qt_sb[:, et, ls * 512:(ls + 1) * 512],
                                    start=(et == 0),
                                    stop=(et == NT - 1),
                                )
                            nc.scalar.activation(
                                pT[:, st, ls * 512:(ls + 1) * 512],
                                ps[:],
                                mybir.ActivationFunctionType.Exp,
                                scale=SCALE,
                            )

                    # numerator^T [d, l] = V^T-tiles contracted with p
                    for dt in range(NT):
                        v_sb = vpool.tile([128, NT, 128], F32R, tag="v")
                        nc.sync.dma_start(
                            v_sb[:, 0:8, :], vc_r[:, :, dt * 128:(dt + 1) * 128]
                        )
                        nc.sync.dma_start(
                            v_sb[:, 8:NT, :], v_dr[:, :, dt * 128:(dt + 1) * 128]
                        )
                        for ls in range(2):
                            ps_o = psO.tile([128, 512], F32, tag="psO")
                            for st in range(NT):
                                nc.tensor.matmul(
                                    ps_o[:],
                                    v_sb[:, st, :],
                                    pT[:, st, ls * 512:(ls + 1) * 512],
                                    start=(st == 0),
                                    stop=(st == NT - 1),
                                )
                            o_sb = oApool.tile([128, 512], F32, tag="o")
                            nc.vector.tensor_copy(o_sb[:], ps_o[:])
                            nc.sync.dma_start(
                                outT[dt * 128:(dt + 1) * 128,
                                     lo + ls * 512:lo + (ls + 1) * 512],
                                o_sb[:],
                            )

                    # denominator [1, l] = ones^T @ p
                    for ls in range(2):
                        ps_d = psD.tile([1, 512], F32, tag="psD")
                        for st in range(NT):
                            nc.tensor.matmul(
                                ps_d[:],
                                ones[:],
                                pT[:, st, ls * 512:(ls + 1) * 512],
                                start=(st == 0),
                                stop=(st == NT - 1),
                            )
                        d_sb = oApool.tile([1, 512], F32, tag="d")
                        nc.vector.tensor_copy(d_sb[:], ps_d[:])
                        nc.sync.dma_start(
                            den[0:1, lo + ls * 512:lo + (ls + 1) * 512], d_sb[:]
                        )


def make_in_maps(x, cache_k, cache_v, Wq, Wk, Wv):
    """Per-core input maps for the SPMD launch. Core c = (b, h) with
    b = c // 2, h = c % 2."""
    f32 = np.float32
    wqT = np.ascontiguousarray(np.asarray(Wq, f32).T)
    wkT = np.ascontiguousarray(np.asarray(Wk, f32).T)
    wvT = np.ascontiguousarray(np.asarray(Wv, f32).T)
    in_maps = []
    for c in range(N_CORES):
        b, h = divmod(c, 2)
        xb = np.asarray(x[b], f32)
        sl = slice(h * HALF, (h + 1) * HALF)
        in_maps.append({
            "xT": np.ascontiguousarray(xb.T),
            "xkvT": np.ascontiguousarray(xb[sl].T),
            "wqT": wqT,
            "wkT": wkT,
            "wvT": wvT,
            "kcT": np.ascontiguousarray(np.asarray(cache_k[b, sl], f32).T),
            "vc": np.ascontiguousarray(np.asarray(cache_v[b, sl], f32)),
        })
    return in_maps


def combine(results):
    """Host combine: out[b] = ((numT_h0 + numT_h1) / (den_h0 + den_h1)).T"""
    B = N_CORES // 2
    out = np.empty((B, L, D), np.float32)
    for b in range(B):
        r0, r1 = results[2 * b], results[2 * b + 1]
        num = r0["outT"].astype(np.float64) + r1["outT"].astype(np.float64)
        dent = r0["den"][0].astype(np.float64) + r1["den"][0].astype(np.float64)
        out[b] = (num / dent[None, :]).T.astype(np.float32)
    return out


def kernel(x, cache_k, cache_v, Wq, Wk, Wv):
    nc = build_program()
    in_maps = make_in_maps(x, cache_k, cache_v, Wq, Wk, Wv)
    results = bass2jax.run_bass_via_pjrt(nc, in_maps, n_cores=N_CORES)
    return combine(results)



# revision 2
# speedup vs baseline: 1.0385x; 1.0385x over previous
"""Cached-attention kernel v2 for Trainium2 (8 NeuronCores, Bass/Tile).

Problem: B=4, L=2048 new tokens, S=2048 cached tokens, D=2048.
  Q = x @ Wq.T ; K = x @ Wk.T ; V = x @ Wv.T
  K_cal = concat(K, cache_k) ; V_cal = concat(V, cache_v)
  out = softmax(Q @ K_cal.T / sqrt(D)) @ V_cal

Sharding: 8 cores = (batch b in 0..3) x (key-half h in 0..1). Each core
handles ALL queries of its batch against HALF the keys (1024 cached +
1024 new); per-core K/V projections cover only its half of the new
tokens. Un-normalized numerator [L, D] and denominator [L] are
returned; the host combines the two halves exactly.

v2 vs v1:
- all matmul operands are bf16 (same PE rate as fp32r, half the
  DMA/SBUF traffic);
- Q^T and V_new stay resident in SBUF, only K^T_new round-trips
  through DRAM; cached K/V stream per query-half;
- numerator is produced in [l, d] layout so no host transpose and no
  partial-sum transpose are needed.
"""

import numpy as np
import ml_dtypes

import concourse.bass as bass
import concourse.tile as tile
from concourse import bacc, mybir
from concourse import bass2jax

F32 = mybir.dt.float32
BF16 = mybir.dt.bfloat16

D = 2048          # model dim (= projection output dim)
L = 2048          # new tokens (queries)
HALF = 1024       # per-core share of new tokens / cached tokens
NT = D // 128     # 16 tiles of 128 along D/E/L
SCALE = 1.0 / float(np.sqrt(D))
N_CORES = 8

_NC_CACHE = {}


def build_program(reps=1):
    key = ("nc", reps)
    if key in _NC_CACHE:
        return _NC_CACHE[key]
    nc = bacc.Bacc(None, target_bir_lowering=False, debug=False)
    xT = nc.dram_tensor("xT", [D, L], BF16, kind="ExternalInput")
    wqT = nc.dram_tensor("wqT", [D, D], BF16, kind="ExternalInput")
    wkT = nc.dram_tensor("wkT", [D, D], BF16, kind="ExternalInput")
    wvT = nc.dram_tensor("wvT", [D, D], BF16, kind="ExternalInput")
    kcT = nc.dram_tensor("kcT", [D, HALF], BF16, kind="ExternalInput")
    vc = nc.dram_tensor("vc", [HALF, D], BF16, kind="ExternalInput")
    outT = nc.dram_tensor("outT", [L, D], F32, kind="ExternalOutput")
    den = nc.dram_tensor("den", [1, L], F32, kind="ExternalOutput")

    from contextlib import ExitStack
    with tile.TileContext(nc) as tc:
        with ExitStack() as _rep_stack:
            if reps > 1:
                _rep_stack.enter_context(
                    tc.For_i(0, reps, 1, hint_engines=tuple(mybir.EngineType))
                )
            _emit_body(nc, tc, xT, wqT, wkT, wvT, kcT, vc, outT, den)
    nc.compile()
    _NC_CACHE[key] = nc
    return nc


def _emit_body(nc, tc, xT, wqT, wkT, wvT, kcT, vc, outT, den):
    # host always places the core's own kv-half at xT columns [0, HALF)
    xT_r = xT.rearrange("(t p) l -> p t l", p=128)
    wqT_r = wqT.rearrange("(t p) e -> p t e", p=128)
    wkT_r = wkT.rearrange("(t p) e -> p t e", p=128)
    wvT_r = wvT.rearrange("(t p) e -> p t e", p=128)
    kcT_r = kcT.rearrange("(t p) s -> p t s", p=128)
    vc_r = vc.rearrange("(t p) d -> p t d", p=128)

    with (
        tc.tile_pool(name="dram", bufs=1, space="DRAM") as dpool,
        tc.tile_pool(name="persist", bufs=1) as perpool,
    ):
        kt_d = dpool.tile([D, HALF], BF16, tag="ktD")
        kt_dr = kt_d[:].rearrange("(t p) s -> p t s", p=128)
        QT_sb = perpool.tile([128, NT, L], BF16, tag="QT")
        V_sb = perpool.tile([128, HALF // 128, D], BF16, tag="V")

        # ---------- projections ----------
        with (
            tc.tile_pool(name="xt", bufs=2) as xpool,
            tc.tile_pool(name="w", bufs=2) as wpool,
            tc.tile_pool(name="wv", bufs=2) as wvpool,
            tc.tile_pool(name="oP", bufs=4) as oPpool,
            tc.tile_pool(name="psP", bufs=4, space="PSUM") as psP,
        ):
            for xpass in range(2):
                xt = xpool.tile([128, NT, HALF], BF16, tag="xt")
                nc.sync.dma_start(
                    xt[:], xT_r[:, :, xpass * HALF:(xpass + 1) * HALF])

                if xpass == 0:
                    # K^T_new[e, s] = Wk @ x_kv^T  -> DRAM scratch
                    for et in range(NT):
                        w_sb = wpool.tile([128, NT, 128], BF16, tag="w")
                        nc.sync.dma_start(
                            w_sb[:], wkT_r[:, :, et * 128:(et + 1) * 128])
                        for sc in range(2):
                            ps = psP.tile([128, 512], F32, tag="ps")
                            for dt in range(NT):
                                nc.tensor.matmul(
                                    ps[:],
                                    w_sb[:, dt, :],
                                    xt[:, dt, sc * 512:(sc + 1) * 512],
                                    start=(dt == 0),
                                    stop=(dt == NT - 1),
                                )
                            o_sb = oPpool.tile([128, 512], BF16, tag="ko")
                            nc.vector.tensor_copy(o_sb[:], ps[:])
                            nc.sync.dma_start(
                                kt_d[et * 128:(et + 1) * 128,
                                     sc * 512:(sc + 1) * 512],
                                o_sb[:],
                            )

                    # V_new[s, d] = x_kv @ Wv^T  -> SBUF resident
                    for dcq in range(4):
                        wv_sb = wvpool.tile([128, NT, 512], BF16, tag="wv")
                        nc.sync.dma_start(
                            wv_sb[:], wvT_r[:, :, dcq * 512:(dcq + 1) * 512])
                        for st in range(8):
                            ps = psP.tile([128, 512], F32, tag="ps")
                            for dt in range(NT):
                                nc.tensor.matmul(
                                    ps[:],
                                    xt[:, dt, st * 128:(st + 1) * 128],
                                    wv_sb[:, dt, :],
                                    start=(dt == 0),
                                    stop=(dt == NT - 1),
                                )
                            nc.vector.tensor_copy(
                                V_sb[:, st, dcq * 512:(dcq + 1) * 512], ps[:])

                # Q^T[e, l] = Wq @ x^T for this l-half -> SBUF resident
                for et in range(NT):
                    w_sb = wpool.tile([128, NT, 128], BF16, tag="w")
                    nc.sync.dma_start(
                        w_sb[:], wqT_r[:, :, et * 128:(et + 1) * 128])
                    for lc in range(2):
                        ps = psP.tile([128, 512], F32, tag="ps")
                        for dt in range(NT):
                            nc.tensor.matmul(
                                ps[:],
                                w_sb[:, dt, :],
                                xt[:, dt, lc * 512:(lc + 1) * 512],
                                start=(dt == 0),
                                stop=(dt == NT - 1),
                            )
                        nc.vector.tensor_copy(
                            QT_sb[:, et,
                                  xpass * HALF + lc * 512:
                                  xpass * HALF + (lc + 1) * 512],
                            ps[:])

        # ---------- attention ----------
        with (
            tc.tile_pool(name="pt", bufs=1) as ptpool,
            tc.tile_pool(name="kst", bufs=4) as kpool,
            tc.tile_pool(name="dacc", bufs=2) as dapool,
            tc.tile_pool(name="vcsb", bufs=1) as vcpool,
            tc.tile_pool(name="oA", bufs=4) as oApool,
            tc.tile_pool(name="cst", bufs=1) as cpool,
        ):
            ones_f = cpool.tile([128, 1], F32, tag="ones_f")
            nc.gpsimd.memset(ones_f[:], 1.0)
            ones = cpool.tile([128, 1], BF16, tag="ones")
            nc.vector.tensor_copy(ones[:], ones_f[:])

            pT = ptpool.tile([128, NT, HALF], BF16, tag="pT")

            for lc2 in range(2):
                lo = lc2 * HALF

                # cached V for the numerator: load early, overlaps scores
                vc_sb = vcpool.tile([128, HALF // 128, D], BF16, tag="vc")
                nc.sync.dma_start(vc_sb[:], vc_r[:, :, :])

                # scores^T [s, l] and p = exp(scale * s)
                with (
                    tc.tile_pool(name="psS", bufs=4, space="PSUM") as psS,
                    tc.tile_pool(name="psD", bufs=2, space="PSUM") as psD,
                ):
                    for st in range(NT):
                        ksrc = kpool.tile([128, NT, 128], BF16, tag="k")
                        if st < 8:
                            src = kcT_r[:, :, st * 128:(st + 1) * 128]
                        else:
                            src = kt_dr[:, :, (st - 8) * 128:(st - 7) * 128]
                        nc.sync.dma_start(ksrc[:], src)
                        for lq in range(2):
                            ps = psS.tile([128, 512], F32, tag="psS")
                            for dt in range(NT):
                                nc.tensor.matmul(
                                    ps[:],
                                    ksrc[:, dt, :],
                                    QT_sb[:, dt,
                                          lo + lq * 512:lo + (lq + 1) * 512],
                                    start=(dt == 0),
                                    stop=(dt == NT - 1),
                                )
                            nc.scalar.activation(
                                pT[:, st, lq * 512:(lq + 1) * 512],
                                ps[:],
                                mybir.ActivationFunctionType.Exp,
                                scale=SCALE,
                            )

                    # denominator: DVE tree over st, then a single
                    # ones-stationary matmul for the partition reduction
                    dacc = dapool.tile([128, HALF], F32, tag="dacc")
                    nc.vector.tensor_tensor(
                        dacc[:], pT[:, 0, :], pT[:, 1, :],
                        mybir.AluOpType.add)
                    for st in range(2, NT):
                        nc.vector.tensor_tensor(
                            dacc[:], dacc[:], pT[:, st, :],
                            mybir.AluOpType.add)
                    for ls in range(2):
                        ps_d = psD.tile([1, 512], F32, tag="psD")
                        nc.tensor.matmul(
                            ps_d[:],
                            ones_f[:],
                            dacc[:, ls * 512:(ls + 1) * 512],
                            start=True,
                            stop=True,
                        )
                        d_sb = oApool.tile([1, 512], F32, tag="d")
                        nc.vector.tensor_copy(d_sb[:], ps_d[:])
                        nc.sync.dma_start(
                            den[0:1, lo + ls * 512:lo + (ls + 1) * 512],
                            d_sb[:])

                # numerator [l, d] = p^T-tiles (stationary) x V (moving)
                with tc.tile_pool(name="psN", bufs=8, space="PSUM") as psN:
                    for lt in range(8):
                        pss = [psN.tile([128, 512], F32, tag="psN",
                                        name=f"psN_{lc2}_{lt}_{i}")
                               for i in range(4)]
                        for st in range(NT):
                            if st < 8:
                                vsrc = vc_sb
                                sti = st
                            else:
                                vsrc = V_sb
                                sti = st - 8
                            for dc in range(4):
                                nc.tensor.matmul(
                                    pss[dc][:],
                                    pT[:, st, lt * 128:(lt + 1) * 128],
                                    vsrc[:, sti, dc * 512:(dc + 1) * 512],
                                    start=(st == 0),
                                    stop=(st == NT - 1),
                                )
                        for dc in range(4):
                            o_sb = oApool.tile([128, 512], F32, tag="o")
                            nc.vector.tensor_copy(o_sb[:], pss[dc][:])
                            nc.sync.dma_start(
                                outT[lo + lt * 128:lo + (lt + 1) * 128,
                                     dc * 512:(dc + 1) * 512],
                                o_sb[:],
                            )


def make_in_maps(x, cache_k, cache_v, Wq, Wk, Wv):
    """Per-core input maps. Core c = (b, h) with b = c // 2, h = c % 2.
    xT columns are rolled so the core's own kv-half is first."""
    bf16 = ml_dtypes.bfloat16
    wqT = np.ascontiguousarray(np.asarray(Wq, np.float32).T.astype(bf16))
    wkT = np.ascontiguousarray(np.asarray(Wk, np.float32).T.astype(bf16))
    wvT = np.ascontiguousarray(np.asarray(Wv, np.float32).T.astype(bf16))
    in_maps = []
    for c in range(N_CORES):
        b, h = divmod(c, 2)
        xbT = np.asarray(x[b], np.float32).astype(bf16).T  # [D, L]
        if h == 1:
            xbT = np.concatenate([xbT[:, HALF:], xbT[:, :HALF]], axis=1)
        sl = slice(h * HALF, (h + 1) * HALF)
        in_maps.append({
            "xT": np.ascontiguousarray(xbT),
            "wqT": wqT,
            "wkT": wkT,
            "wvT": wvT,
            "kcT": np.ascontiguousarray(
                np.asarray(cache_k[b, sl], np.float32).T.astype(bf16)),
            "vc": np.ascontiguousarray(
                np.asarray(cache_v[b, sl], np.float32).astype(bf16)),
        })
    return in_maps


def combine(results):
    """Host combine: out[b] = (num_h0 + num_h1) / (den_h0 + den_h1).
    Core h=1 computed Q from rolled x columns, so its numerator/den rows
    are rolled too; unroll before adding."""
    B = N_CORES // 2
    out = np.empty((B, L, D), np.float32)
    for b in range(B):
        r0, r1 = results[2 * b], results[2 * b + 1]
        n0 = r0["outT"].astype(np.float64)
        d0 = r0["den"][0].astype(np.float64)
        n1 = np.roll(r1["outT"].astype(np.float64), HALF, axis=0)
        d1 = np.roll(r1["den"][0].astype(np.float64), HALF, axis=0)
        out[b] = ((n0 + n1) / (d0 + d1)[:, None]).astype(np.float32)
    return out


def kernel(x, cache_k, cache_v, Wq, Wk, Wv):
    nc = build_program()
    in_maps = make_in_maps(x, cache_k, cache_v, Wq, Wk, Wv)
    results = bass2jax.run_bass_via_pjrt(nc, in_maps, n_cores=N_CORES)
    return combine(results)
